# revision 76
# baseline (speedup 1.0000x reference)
"""Multi-head attention (B=4, H=16, S=1024, D=64) on 8 Trainium2 cores.

Sharding: core c -> batch b = c//2, head-half hh = c%2 (8 heads each).
Per-core kernel computes, for each head h:
    S_T[k, q] = sum_d K[k, d] Q[q, d]          (scores, transposed layout)
    E = exp(S_T / 8)                           (ScalarE, reads PSUM)
    P_T = E * mask_T                           (VectorE; mask_T is 0/1 f32)
    ctx_T[d|sum, q] = [V | 1]^T-style matmul   (V natural layout as lhsT,
                                                ones column gives row sums)
    out[q, d] = transpose(ctx_T)[q, d] / rowsum[q]
"""

import numpy as np

import concourse.bass as bass
import concourse.mybir as mybir
import concourse.tile as tile
from concourse import bacc
from concourse.bass_utils import run_bass_kernel_spmd
from concourse.masks import make_identity

F32 = mybir.dt.float32
BF16 = mybir.dt.bfloat16
I32 = mybir.dt.int32

S = 1024          # sequence length
DH = 64           # head dim
HEADS = 8         # heads per core
DCORE = HEADS * DH  # 512, model-dim slice per core
NQT = S // 128    # 8 q blocks
NKT = S // 128    # 8 k blocks
SCALE = 1.0 / 8.0  # 1/sqrt(64)
MASK_BIG = 16384.0  # exact in bf16; 0.125*16384 = 2048 exact in f32
N_INJECT = 0      # heads that inject the mask via PE (no mask_T dependency)


def build_nc(reps: int = 1, loop: int = 0):
    nc = bacc.Bacc(
        "TRN2", target_bir_lowering=False, debug=False, num_devices=8
    )
    q_in = nc.declare_dram_parameter("q", [S, DCORE], F32, isOutput=False)
    k_in = nc.declare_dram_parameter("k", [S, DCORE], F32, isOutput=False)
    v_in = nc.declare_dram_parameter("v", [S, DCORE], F32, isOutput=False)
    m_in = nc.declare_dram_parameter("mask", [S, S], I32, isOutput=False)
    out = nc.declare_dram_parameter("out", [S, DCORE], F32, isOutput=True)

    with tile.TileContext(nc) as tc:
        with (
            tc.tile_pool(name="persist", bufs=1) as persist,
            tc.tile_pool(name="stage", bufs=16) as stage,
            tc.tile_pool(name="mstage", bufs=2) as mstage,
            tc.tile_pool(name="work", bufs=4) as work,
            tc.tile_pool(name="vload", bufs=2) as vload,
            tc.tile_pool(name="outp", bufs=4) as outp,
            tc.tile_pool(name="ps_s", bufs=2, space="PSUM") as ps_s_pool,
            tc.tile_pool(name="ps_ctx", bufs=1, space="PSUM") as ps_ctx_pool,
            tc.tile_pool(name="ps_t", bufs=2, space="PSUM") as ps_t_pool,
        ):
            ident = persist.tile([128, 128], F32, tag="ident")
            make_identity(nc, ident[:])
            ident_big = persist.tile([128, 128], BF16, tag="identb")
            nc.scalar.mul(ident_big[:], ident[:], MASK_BIG)
            ident_b16 = persist.tile([128, 128], BF16, tag="identb16")
            nc.vector.tensor_copy(ident_b16[:], ident[:])
            neg_big = persist.tile([128, 1], F32, tag="negbig")
            nc.vector.memset(neg_big[:], -MASK_BIG * SCALE)

            # kt-major [128k, kt*1024 + q]
            maskT = persist.tile([128, NKT * S], BF16, tag="maskT")
            # one tile per head-pair (dp): the first head pair's matmuls
            # only wait on their own quarter of the transposes
            QT = [
                persist.tile([128, S], BF16, tag=f"QT{dp}", name=f"QT{dp}")
                for dp in range(4)
            ]
            KT = [
                persist.tile([128, S], BF16, tag=f"KT{dp}", name=f"KT{dp}")
                for dp in range(4)
            ]

            def body():
                emit_body(
                    nc, tc, q_in, k_in, v_in, m_in, out, ident, ident_big,
                    ident_b16, neg_big, maskT, QT, KT,
                    persist, stage, mstage, work, vload, outp,
                    ps_s_pool, ps_ctx_pool, ps_t_pool,
                )

            if loop:
                with tc.For_i(0, loop, 1):
                    for _ in range(reps):
                        body()
            else:
                for _ in range(reps):
                    body()
    nc.compile()
    return nc


def emit_body(
    nc, tc, q_in, k_in, v_in, m_in, out, ident, ident_big, ident_b16, neg_big,
    maskT, QT, KT,
    persist, stage, mstage, work, vload, outp, ps_s_pool, ps_ctx_pool, ps_t_pool,
):
    if True:
        if True:
            # ---- load Q/K rows with casting DMA (SWDGE), then wide-transpose
            # each [128, 512] row block into QT/KT (one xbar op per block:
            # out[:, dp, :] = in[:, dp*128:(dp+1)*128].T) ----
            q16 = [
                persist.tile([128, DCORE], BF16, tag=f"q16_{sb}", name=f"q16_{sb}")
                for sb in range(S // 128)
            ]
            k16 = [
                persist.tile([128, DCORE], BF16, tag=f"k16_{sb}", name=f"k16_{sb}")
                for sb in range(S // 128)
            ]
            mf = [
                persist.tile([128, S], BF16, tag=f"mf{qb}", name=f"mf{qb}")
                for qb in range(NQT)
            ]
            # f32 loads on the SP HWDGE ring (pipelines well; scalar-ring
            # dispatches would occupy the ACT sequencer ahead of the exp
            # stream), bf16 convert on DVE; mask via casting SWDGE loads
            for src, dst16 in ((q_in, q16), (k_in, k16)):
                for sb in range(S // 128):
                    st = stage.tile([128, DCORE], F32, tag="stage")
                    nc.sync.dma_start(st[:], src[sb * 128:(sb + 1) * 128, :])
                    nc.vector.tensor_copy(dst16[sb][:], st[:])
            # Q/K transposes on the PE (idle during the prefix): dp-outer so
            # the first head pair's QT/KT land first
            for dp in range(4):
                for dst16, dstT in ((q16, QT[dp]), (k16, KT[dp])):
                    for sb in range(S // 128):
                        ps_q = ps_t_pool.tile([128, 128], BF16, tag="t")
                        nc.tensor.transpose(
                            ps_q[:],
                            dst16[sb][:, dp * 128:(dp + 1) * 128],
                            ident_b16[:],
                        )
                        nc.vector.tensor_copy(
                            dstT[:, sb * 128:(sb + 1) * 128], ps_q[:]
                        )
            # mask transposes MUST stay on the scalar ring: moving them to
            # the sync ring (mixed with DMACopies) corrupts results — likely
            # the DMATranspose/DMACopy xbar-mode transition hazard
            for qb in range(NQT):
                nc.gpsimd.dma_start(mf[qb][:], m_in[qb * 128:(qb + 1) * 128, :])
                nc.scalar.dma_start_transpose(
                    maskT[:].rearrange("p (kt q) -> p kt q", q=S)[
                        :, :, qb * 128:(qb + 1) * 128
                    ],
                    mf[qb][:],
                )

            # ---- per-head attention ----
            for h in range(HEADS):
                dp = h // 2
                hp = (h % 2) * 64  # partition base of this head inside the pair tile

                # V_aug [128k, kt, 65] bf16 (col 64 = ones): one casting DMA
                v_t = vload.tile([128, NKT, 65], BF16, tag="v")
                nc.gpsimd.dma_start(
                    v_t[:, :, 0:64],
                    v_in[:, h * 64:(h + 1) * 64].rearrange(
                        "(kt p) d -> p kt d", p=128
                    ),
                )
                nc.vector.memset(v_t[:, :, 64:65], 1.0)

                inject = h < N_INJECT
                ps_ctx = ps_ctx_pool.tile([65, S], F32, tag="ctx")
                for kt in range(NKT):
                    ps_s = ps_s_pool.tile([128, S], F32, tag="s")
                    for qc in range(2):
                        nc.tensor.matmul(
                            ps_s[:, qc * 512:(qc + 1) * 512],
                            lhsT=KT[dp][
                                hp:hp + 64, kt * 128:(kt + 1) * 128
                            ],
                            rhs=QT[dp][
                                hp:hp + 64, qc * 512:(qc + 1) * 512
                            ],
                            start=True,
                            stop=not inject,
                        )
                    if inject:
                        # scores += MASK_BIG * mask_T, via native-layout mask
                        # as lhsT against a scaled identity (contract over q)
                        for qb in range(NQT):
                            nc.tensor.matmul(
                                ps_s[:, qb * 128:(qb + 1) * 128],
                                lhsT=mf[qb][:, kt * 128:(kt + 1) * 128],
                                rhs=ident_big[:],
                                start=False,
                                stop=(qb % 4 == 3),  # last write per PSUM bank
                            )
                    e_sb = work.tile([128, S], BF16, tag="e")
                    nc.scalar.activation(
                        e_sb[:],
                        ps_s[:],
                        mybir.ActivationFunctionType.Exp,
                        scale=SCALE,
                        bias=neg_big[:] if inject else 0.0,
                    )
                    if inject:
                        p_sb = e_sb
                    else:
                        p_sb = work.tile([128, S], BF16, tag="p")
                        nc.vector.tensor_mul(
                            p_sb[:], e_sb[:], maskT[:, kt * S:(kt + 1) * S]
                        )
                    for qc in range(2):
                        nc.tensor.matmul(
                            ps_ctx[:, qc * 512:(qc + 1) * 512],
                            lhsT=v_t[:, kt, :],
                            rhs=p_sb[:, qc * 512:(qc + 1) * 512],
                            start=(kt == 0),
                            stop=(kt == NKT - 1),
                        )

                ctxT = work.tile([65, S], F32, tag="ctxT")
                nc.vector.tensor_copy(ctxT[:], ps_ctx[:])
                o_head = outp.tile([128, NQT, 64], F32, tag="o")
                for qb in range(NQT):
                    ps_o = ps_t_pool.tile([128, 65], F32, tag="t")
                    nc.tensor.transpose(
                        ps_o[:], ctxT[:, qb * 128:(qb + 1) * 128], ident[:65, :65]
                    )
                    recip = outp.tile([128, 1], F32, tag="r")
                    nc.vector.reciprocal(recip[:], ps_o[:, 64:65])
                    nc.vector.tensor_scalar_mul(
                        o_head[:, qb, :], ps_o[:, 0:64], recip[:]
                    )
                nc.sync.dma_start(
                    out[:, h * 64:(h + 1) * 64].rearrange("(qb p) d -> p qb d", p=128),
                    o_head[:],
                )


_NC_CACHE = None


def kernel(query, key, value, attention_mask):
    global _NC_CACHE
    query = np.asarray(query, dtype=np.float32)
    key = np.asarray(key, dtype=np.float32)
    value = np.asarray(value, dtype=np.float32)
    attention_mask = np.asarray(attention_mask, dtype=np.int32)

    B = query.shape[0]
    in_maps = []
    for c in range(8):
        b, hh = c // 2, c % 2
        sl = slice(hh * DCORE, (hh + 1) * DCORE)
        in_maps.append(
            {
                "q": np.ascontiguousarray(query[b, :, sl]),
                "k": np.ascontiguousarray(key[b, :, sl]),
                "v": np.ascontiguousarray(value[b, :, sl]),
                "mask": np.ascontiguousarray(attention_mask[b]),
            }
        )

    if _NC_CACHE is None:
        _NC_CACHE = build_nc()
    res = run_bass_kernel_spmd(_NC_CACHE, in_maps, core_ids=list(range(8)))

    outf = np.empty((B, S, 2 * DCORE), np.float32)
    for c in range(8):
        b, hh = c // 2, c % 2
        outf[b, :, hh * DCORE:(hh + 1) * DCORE] = res.results[c]["out"]
    return outf


if __name__ == "__main__":
    rng = np.random.default_rng(0)
    q = rng.standard_normal((4, S, 1024), dtype=np.float32)
    k = rng.standard_normal((4, S, 1024), dtype=np.float32)
    v = rng.standard_normal((4, S, 1024), dtype=np.float32)
    m = rng.integers(0, 2, size=(4, S, S)).astype(np.int32)
    o = kernel(q, k, v, m)
    print(o.shape, o.dtype)
